# revision 1
# baseline (speedup 1.0000x reference)
"""Trainium2 Bass kernel for nn_CovarianceLayer.

Math: x = inputs[:,0,:] + i*inputs[:,1,:]  (B=256 complex signals, N=1024)
      hankel[b,i,j] = x[b,(j+i)%N]  (L=128 rolled copies)
      out[b,l,m,0]  = Re( hankel @ hankel^H )[l,m] / L
                    = (1/L) * sum_n ( Hr[l,n]Hr[m,n] + Hi[l,n]Hi[m,n] )

Per-core strategy (pure data parallel, 32 batches per core):
  - wrap-pad + cast x to fp16 once into a DRAM scratch [32,2,N+L]
  - per batch: one overlapping-AP DMA builds the Hankel tile
    TT[p, j] = xdup[p + j] directly in SBUF (TT[:, 128t:128(t+1)] is the
    t-th K-chunk of the Hankel, transposed, i.e. exactly the matmul operand)
  - 16 accumulating 128x128 matmuls (8 n-chunks x {real,imag}) into PSUM
  - scale by 1/L on ScalarE, DMA out
"""

import os

import numpy as np

import concourse.bacc as bacc
import concourse.mybir as mybir
import concourse.tile as tile
from concourse.bass_types import AP
from concourse.bass_utils import run_bass_kernel_spmd

B, L, N = 256, 128, 1024
NCORES = 8
BPC = B // NCORES  # 32 batches per core
NPAD = N + L  # 1152

_CACHE = {}
LAST_RESULT = None  # BassKernelResults of the most recent run (for test.py)


def build_nc(mm_dtype=mybir.dt.float16):
    nc = bacc.Bacc(
        "TRN2", target_bir_lowering=False, debug=False, num_devices=NCORES
    )
    inp = nc.dram_tensor("inp", [BPC, 2, N], mybir.dt.float32, kind="ExternalInput")
    out = nc.dram_tensor("out", [BPC, L, L], mybir.dt.float32, kind="ExternalOutput")

    with tile.TileContext(nc) as tc:
        with (
            tc.tile_pool(name="prep", bufs=1) as prep_pool,
            tc.tile_pool(name="dram", bufs=1, space="DRAM") as dram_pool,
            tc.tile_pool(name="hank", bufs=4) as hank_pool,
            tc.tile_pool(name="osb", bufs=4) as opool,
            tc.tile_pool(name="psum", bufs=4, space="PSUM") as ppool,
        ):
            # --- one-time prep: wrap-pad + cast to fp16, park in DRAM ---
            xsb = prep_pool.tile([2 * BPC, NPAD], mybir.dt.float32)
            flat_in = inp[:].rearrange("b c n -> (b c) n")
            nc.sync.dma_start(out=xsb[:, 0:N], in_=flat_in)
            nc.sync.dma_start(out=xsb[:, N:NPAD], in_=flat_in[:, 0:L])
            xhb = prep_pool.tile([2 * BPC, NPAD], mm_dtype)
            nc.vector.tensor_copy(xhb[:], xsb[:])
            xdup = dram_pool.tile([2 * BPC, NPAD], mm_dtype)
            nc.sync.dma_start(out=xdup[:], in_=xhb[:])

            # --- per-batch Hankel Gram ---
            for b in range(BPC):
                tt = hank_pool.tile([128, 2 * N], mm_dtype)
                for c in range(2):
                    src = AP(
                        tensor=xdup.tensor,
                        offset=xdup.offset + (2 * b + c) * NPAD,
                        ap=[[1, 128], [1, N]],
                    )
                    nc.sync.dma_start(out=tt[:, c * N : (c + 1) * N], in_=src)
                ps = ppool.tile([128, 128], mybir.dt.float32)
                for k in range(16):
                    c, t = divmod(k, 8)
                    sl = tt[:, c * N + t * 128 : c * N + t * 128 + 128]
                    nc.tensor.matmul(
                        ps[:], sl, sl, start=(k == 0), stop=(k == 15)
                    )
                ob = opool.tile([128, 128], mybir.dt.float32)
                nc.scalar.mul(ob[:], ps[:], 1.0 / L)
                nc.sync.dma_start(out=out[b], in_=ob[:])

    nc.compile()
    return nc


def kernel(inputs: np.ndarray) -> np.ndarray:
    global LAST_RESULT
    inputs = np.ascontiguousarray(np.asarray(inputs), dtype=np.float32)
    assert inputs.shape == (B, 2, N), inputs.shape

    if "nc" not in _CACHE:
        _CACHE["nc"] = build_nc()
    nc = _CACHE["nc"]

    in_maps = [{"inp": inputs[c * BPC : (c + 1) * BPC]} for c in range(NCORES)]
    # NTFF tracing needs hooks this container lacks; always run untraced.
    res = run_bass_kernel_spmd(nc, in_maps, list(range(NCORES)), trace=False)
    LAST_RESULT = res
    outf = np.concatenate([res.results[c]["out"] for c in range(NCORES)], axis=0)
    return outf.reshape(B, L, L, 1).astype(np.float32, copy=False)



# revision 4
# speedup vs baseline: 3.5046x; 3.5046x over previous
"""Trainium2 Bass kernel for nn_CovarianceLayer (Toeplitz-autocorrelation form).

Math: x = inputs[:,0,:] + i*inputs[:,1,:]  (B=256 complex signals, N=1024)
      cov[b,l,m] = Re(hankel @ hankel^H)[l,m] / L  with hankel[b,i,j] = x[b,(j+i)%N]
By circularity cov[b,l,m] = r_b[|l-m|] / L where
      r_b[d] = sum_n ( xr[n]xr[n+d] + xi[n]xi[n+d] )   (indices mod N)
i.e. each [L,L] output tile is a symmetric Toeplitz matrix built from a
128-point autocorrelation.

Per-core plan (32 batches/core, pure data parallel):
  - 2 gpsimd casting DMAs build a wrap-padded fp8 copy of x in DRAM
    (xdup row per batch: [x0|pad|x1|pad], 2*1152 elems).
  - per 8-batch group: 2 DMAs build a packed Hankel tile
    Hg[(16c+p), j*1136+u] = x_c[b_j, p+u]; 32 DoubleRow fp8 matmuls per
    batch (K=32 contracts comps+offsets) accumulate r into psum columns.
  - drain+1/L scale, PE-transpose, palindrome copy s[b,k]=r[|k-127|],
    one mirror DMA to DRAM, then ONE strided DMA per group expands the
    Toeplitz tiles straight into the output:
    out[b,l,m] = s_all[b, 127-l+m]  (contiguous 512B runs both sides).
"""

import numpy as np

import concourse.bacc as bacc
import concourse.mybir as mybir
import concourse.tile as tile
from concourse.bass_types import AP
from concourse.bass_utils import run_bass_kernel_spmd

B, L, N = 256, 128, 1024
NCORES = 8
BPC = B // NCORES  # 32 batches per core

P = 16  # n-offsets per matmul chunk
K = 2 * P  # contraction width (comps folded)
T = N // P  # 64 chunks per batch -> 32 DoubleRow matmuls
W = N - P + 128  # 1136: hankel window elems per partition per batch
CROW = 1152  # padded per-comp row in xdup
ROW = 2 * CROW  # 2304: xdup elems per batch
GB = 8  # batches per pipeline group
NG = BPC // GB  # 4 groups

_CACHE = {}
LAST_RESULT = None


def build_nc():
    f8 = mybir.dt.float8e4
    f32 = mybir.dt.float32
    nc = bacc.Bacc(
        "TRN2", target_bir_lowering=False, debug=False, num_devices=NCORES
    )
    inp = nc.dram_tensor("inp", [BPC, 2, N], f32, kind="ExternalInput")
    out = nc.dram_tensor("out", [BPC, L, L], f32, kind="ExternalOutput")

    with tile.TileContext(nc) as tc:
        with (
            tc.tile_pool(name="const", bufs=1) as cpool,
            tc.tile_pool(name="dram", bufs=1, space="DRAM") as dpool,
            tc.tile_pool(name="hank", bufs=2) as hpool,
            tc.tile_pool(name="spal", bufs=2) as spool,
            tc.tile_pool(name="rr", bufs=2) as rpool,
            tc.tile_pool(name="psum", bufs=2, space="PSUM") as ppool,
            tc.tile_pool(name="pst", bufs=2, space="PSUM") as tpool,
        ):
            # --- identity for PE transpose ---
            ones = cpool.tile([128, 128], f32)
            nc.vector.memset(ones[:], 1.0)
            ident = cpool.tile([128, 128], f32)
            nc.gpsimd.affine_select(
                out=ident[:],
                in_=ones[:],
                pattern=[[1, 128]],
                compare_op=mybir.AluOpType.is_equal,
                fill=0.0,
                base=0,
                channel_multiplier=-1,
            )

            # --- wrap-padded fp8 signal in DRAM via casting DMAs ---
            xdup = dpool.tile([2 * BPC, CROW], f8)  # row (2b+c) = x_c[b] padded
            flat = inp[:].rearrange("b c n -> (b c) n")
            nc.gpsimd.dma_start(out=xdup[:, 0:N], in_=flat)
            nc.gpsimd.dma_start(out=xdup[:, N:CROW], in_=flat[:, 0:128])

            s_all = dpool.tile([BPC, 256], f32)  # palindrome rows per batch

            for g in range(NG):
                # --- packed hankel tile: Hg[16c+p, j*W+u] = x_c[b, p+u] ---
                hg = hpool.tile([K, GB * W], f8)
                base = g * GB * ROW
                for c in range(2):
                    src = AP(
                        tensor=xdup.tensor,
                        offset=xdup.offset + base + c * CROW,
                        ap=[[1, P], [ROW, GB], [1, W]],
                    )
                    eng = nc.sync if c == 0 else nc.scalar
                    eng.dma_start(out=hg[16 * c : 16 * c + 16, :], in_=src)

                # --- autocorrelation matmuls: psum col j accumulates r_b ---
                ps = ppool.tile([128, GB], f32)
                for j in range(GB):
                    col = j * W
                    for tp in range(T // 2):
                        off = col + K * tp  # = 16*(2*tp)
                        lhsT = AP(
                            tensor=hg.tensor,
                            offset=hg.offset + off,
                            ap=[[GB * W, K], [P, 2], [1, 128]],
                        )
                        rhs = AP(
                            tensor=hg.tensor,
                            offset=hg.offset + off,
                            ap=[[GB * W, K], [P, 2], [1, 1]],
                        )
                        nc.tensor.matmul(
                            ps[:, j : j + 1],
                            lhsT,
                            rhs,
                            start=(tp == 0),
                            stop=(tp == T // 2 - 1),
                            perf_mode=mybir.MatmulPerfMode.DoubleRow,
                        )

                # --- drain + 1/L, transpose to [GB, 128] ---
                rg = rpool.tile([128, GB], f32)
                nc.scalar.mul(rg[:], ps[:], 1.0 / L)
                pt = tpool.tile([GB, 128], f32)
                nc.tensor.transpose(pt[:], rg[:], ident[:])

                # --- palindrome rows: s[b, 127+d] = s[b, 127-d] = r_b[d] ---
                rows = spool.tile([GB, 256], f32)
                nc.vector.tensor_copy(rows[:, 127:255], pt[:])
                nc.vector.tensor_copy(rows[:, 0:127], pt[:, 127:0:-1])
                nc.sync.dma_start(
                    out=s_all[g * GB : (g + 1) * GB, :], in_=rows[:]
                )

                # --- Toeplitz expansion: out[b,l,m] = s_all[b, 127-l+m] ---
                src2 = AP(
                    tensor=s_all.tensor,
                    offset=s_all.offset + g * GB * 256 + 127,
                    ap=[[256, GB], [-1, 128], [1, 128]],
                )
                dst2 = AP(
                    tensor=out,
                    offset=g * GB * L * L,
                    ap=[[L * L, GB], [L, 128], [1, 128]],
                )
                nc.scalar.dma_start(out=dst2, in_=src2)

    nc.compile()
    return nc


def kernel(inputs: np.ndarray) -> np.ndarray:
    global LAST_RESULT
    inputs = np.ascontiguousarray(np.asarray(inputs), dtype=np.float32)
    assert inputs.shape == (B, 2, N), inputs.shape

    if "nc" not in _CACHE:
        _CACHE["nc"] = build_nc()
    nc = _CACHE["nc"]

    in_maps = [{"inp": inputs[c * BPC : (c + 1) * BPC]} for c in range(NCORES)]
    res = run_bass_kernel_spmd(nc, in_maps, list(range(NCORES)), trace=False)
    LAST_RESULT = res
    outf = np.concatenate([res.results[c]["out"] for c in range(NCORES)], axis=0)
    return outf.reshape(B, L, L, 1).astype(np.float32, copy=False)


# revision 5
# speedup vs baseline: 4.3755x; 1.2485x over previous
"""Trainium2 Bass kernel for nn_CovarianceLayer (Toeplitz-autocorrelation form).

Math: x = inputs[:,0,:] + i*inputs[:,1,:]  (B=256 complex signals, N=1024)
      cov[b,l,m] = Re(hankel @ hankel^H)[l,m] / L  with hankel[b,i,j] = x[b,(j+i)%N]
By circularity cov[b,l,m] = r_b[|l-m|] / L where
      r_b[d] = sum_n ( xr[n]xr[n+d] + xi[n]xi[n+d] )   (indices mod N)
i.e. each [L,L] output tile is a symmetric Toeplitz matrix built from a
128-point autocorrelation.

Per-core plan (32 batches/core, pure data parallel):
  - 2 gpsimd casting DMAs build a wrap-padded fp8 copy of x in DRAM
    (xdup row per batch: [x0|wrap|x1|wrap], 2*1152 elems).
  - per 8-batch group: 2 DMAs build a packed Hankel tile
    Hg[(16c+p), j*1136+u] = x_c[b_j, p+u]; 32 DoubleRow fp8 matmuls per
    batch (K=32 contracts comps+offsets) accumulate r_b into psum col j.
  - drain+1/L on DVE, PE-transpose, palindrome copy s[b,k]=r_b[|k-127|]
    into an SBUF row per batch, then ONE strided DMA per group expands
    the Toeplitz tiles straight from SBUF into the output:
    out[b,l,m] = s[b, 127-l+m]  (contiguous 512B runs both sides).
All Hankel DMAs are issued up front (4-deep buffers) so transfers,
matmuls, and expansion DMAs of different groups overlap.
"""

import numpy as np

import concourse.bacc as bacc
import concourse.mybir as mybir
import concourse.tile as tile
from concourse.bass_types import AP
from concourse.bass_utils import run_bass_kernel_spmd

B, L, N = 256, 128, 1024
NCORES = 8
BPC = B // NCORES  # 32 batches per core

P = 16  # n-offsets per matmul chunk
K = 2 * P  # contraction width (comps folded)
T = N // P  # 64 chunks per batch -> 32 DoubleRow matmuls
W = N - P + 128  # 1136: hankel window elems per partition per batch
CROW = 1152  # padded per-comp row in xdup
ROW = 2 * CROW  # 2304: xdup elems per batch
GB = 8  # batches per pipeline group
NG = BPC // GB  # 4 groups

_CACHE = {}
LAST_RESULT = None


def build_nc():
    f8 = mybir.dt.float8e4
    f32 = mybir.dt.float32
    nc = bacc.Bacc(
        "TRN2", target_bir_lowering=False, debug=False, num_devices=NCORES
    )
    inp = nc.dram_tensor("inp", [BPC, 2, N], f32, kind="ExternalInput")
    out = nc.dram_tensor("out", [BPC, L, L], f32, kind="ExternalOutput")

    with tile.TileContext(nc) as tc:
        with (
            tc.tile_pool(name="const", bufs=1) as cpool,
            tc.tile_pool(name="dram", bufs=1, space="DRAM") as dpool,
            tc.tile_pool(name="hank", bufs=NG) as hpool,
            tc.tile_pool(name="spal", bufs=NG) as spool,
            tc.tile_pool(name="rr", bufs=NG) as rpool,
            tc.tile_pool(name="psum", bufs=NG, space="PSUM") as ppool,
            tc.tile_pool(name="pst", bufs=NG, space="PSUM") as tpool,
        ):
            # --- identity for PE transpose (overlaps with DMAs) ---
            ones = cpool.tile([128, 128], f32)
            nc.vector.memset(ones[:], 1.0)
            ident = cpool.tile([128, 128], f32)
            nc.gpsimd.affine_select(
                out=ident[:],
                in_=ones[:],
                pattern=[[1, 128]],
                compare_op=mybir.AluOpType.is_equal,
                fill=0.0,
                base=0,
                channel_multiplier=-1,
            )

            # --- wrap-padded fp8 signal in DRAM via casting DMAs ---
            xdup = dpool.tile([2 * BPC, CROW], f8)  # row (2b+c) = x_c[b] padded
            flat = inp[:].rearrange("b c n -> (b c) n")
            nc.gpsimd.dma_start(out=xdup[:, 0:N], in_=flat)
            nc.gpsimd.dma_start(out=xdup[:, N:CROW], in_=flat[:, 0:128])

            # --- all hankel tile DMAs first: Hg[16c+p, j*W+u] = x_c[b_j, p+u]
            hgs = []
            for g in range(NG):
                hg = hpool.tile([K, GB * W], f8)
                hgs.append(hg)
                for c in range(2):
                    src = AP(
                        tensor=xdup.tensor,
                        offset=xdup.offset + g * GB * ROW + c * CROW,
                        ap=[[1, P], [ROW, GB], [1, W]],
                    )
                    eng = nc.sync if (2 * g + c) % 2 == 0 else nc.scalar
                    eng.dma_start(out=hg[16 * c : 16 * c + 16, :], in_=src)

            for g in range(NG):
                hg = hgs[g]
                # --- autocorrelation matmuls: psum col j accumulates r_b ---
                ps = ppool.tile([128, GB], f32)
                for j in range(GB):
                    col = j * W
                    for tp in range(T // 2):
                        off = col + K * tp  # = 16*(2*tp)
                        lhsT = AP(
                            tensor=hg.tensor,
                            offset=hg.offset + off,
                            ap=[[GB * W, K], [P, 2], [1, 128]],
                        )
                        rhs = AP(
                            tensor=hg.tensor,
                            offset=hg.offset + off,
                            ap=[[GB * W, K], [P, 2], [1, 1]],
                        )
                        nc.tensor.matmul(
                            ps[:, j : j + 1],
                            lhsT,
                            rhs,
                            start=(tp == 0),
                            stop=(tp == T // 2 - 1),
                            perf_mode=mybir.MatmulPerfMode.DoubleRow,
                        )

                # --- drain + 1/L on DVE, transpose to [GB, 128] ---
                rg = rpool.tile([128, GB], f32)
                nc.vector.tensor_scalar_mul(rg[:], ps[:], 1.0 / L)
                pt = tpool.tile([GB, 128], f32)
                nc.tensor.transpose(pt[:], rg[:], ident[:])

                # --- palindrome rows: s[b, 127+d] = s[b, 127-d] = r_b[d] ---
                rows = spool.tile([GB, 256], f32)
                nc.vector.tensor_copy(rows[:, 127:255], pt[:])
                nc.vector.tensor_copy(rows[:, 0:127], pt[:, 127:0:-1])

                # --- Toeplitz expansion from SBUF: out[b,l,m] = s[b,127-l+m]
                src2 = AP(
                    tensor=rows.tensor,
                    offset=rows.offset + 127,
                    ap=[[256, GB], [-1, 128], [1, 128]],
                )
                dst2 = AP(
                    tensor=out,
                    offset=g * GB * L * L,
                    ap=[[L * L, GB], [L, 128], [1, 128]],
                )
                eng = nc.sync if g % 2 == 0 else nc.scalar
                eng.dma_start(out=dst2, in_=src2)

    nc.compile()
    return nc


def kernel(inputs: np.ndarray) -> np.ndarray:
    global LAST_RESULT
    inputs = np.ascontiguousarray(np.asarray(inputs), dtype=np.float32)
    assert inputs.shape == (B, 2, N), inputs.shape

    if "nc" not in _CACHE:
        _CACHE["nc"] = build_nc()
    nc = _CACHE["nc"]

    in_maps = [{"inp": inputs[c * BPC : (c + 1) * BPC]} for c in range(NCORES)]
    res = run_bass_kernel_spmd(nc, in_maps, list(range(NCORES)), trace=False)
    LAST_RESULT = res
    outf = np.concatenate([res.results[c]["out"] for c in range(NCORES)], axis=0)
    return outf.reshape(B, L, L, 1).astype(np.float32, copy=False)
